# revision 10
# baseline (speedup 1.0000x reference)
"""Trainium2 Bass kernel for nn_DecoderLayer (B=4,S=T=1024,D=1024,H=16,DFF=4096).

Sharding: row-parallel (sequence-parallel over query rows). Core c owns 512
query rows = half of batch b=c//2 (row offset (c%2)*512). Each core recomputes
K/V for its whole batch -> zero collectives, perfectly uniform SPMD program.
Matmuls in float32r by default (full PE rate, ~1.6e-4 rel err; KDT=bf16 env
switches to bf16). Softmax: exp on ScalarE with fused row-sum (accum_out);
causal mask folded into the PE via an identity-matmul of a host-premultiplied
(-1e9) mask. The PE-side A@V chain consumes UNNORMALIZED exp tiles
(PE-transposed); normalization happens at the A@V eviction via a per-head
1/rowsum broadcast bounced through DRAM. The [q,k]-layout probs are
normalized on DVE and DMA'd out as m_attn/c_attn. LayerNorms row-major via
bn_stats/bn_aggr; out1/out2 spill to DRAM scratch to fit SBUF.
"""
import os
from contextlib import ExitStack

import numpy as np
import ml_dtypes

import concourse.bass as bass
from concourse import bacc
import concourse.mybir as mybir
import concourse.tile as tile
from concourse.bass_utils import run_bass_kernel_spmd

f32 = mybir.dt.float32
f32r = mybir.dt.float32r
bf16 = mybir.dt.bfloat16
AF = mybir.ActivationFunctionType
OP = mybir.AluOpType

B, S, T, D, H, DFF = 4, 1024, 1024, 1024, 16, 4096
DEPTH = D // H          # 64
NC = 8                  # cores
R = (B * S) // NC       # 512 rows per core
RC = R // 128           # 4 row chunks
DC = D // 128           # 8 feature chunks
HC = DFF // 128         # 32 hidden chunks
EPS = 1e-3

KDT = os.environ.get("KDT", "f32r")

_cache = {}


def _build():
    KPH = int(os.environ.get("KPHASES", "7"))
    MDT = bf16 if KDT == "bf16" else f32r
    ATT_DT = bf16 if KDT == "bf16" else f32r
    nc = bacc.Bacc(None, target_bir_lowering=False)

    def din(name, shape, dt=None):
        if dt is None:
            dt = MDT
        return nc.dram_tensor(name, list(shape), dt, kind="ExternalInput")

    xt = din("xt", (128, DC, S))            # x_b^T feature-major chunks
    xtr = din("xtr", (128, DC, R))          # x_rows^T
    xrow = din("xrow", (128, RC, D), f32)   # x rows row-major (residual 1)
    enct = din("enct", (128, DC, T))        # enc_b^T
    mb = din("mb", (128, RC, S), bf16)      # causal mask rows * -1e9 (bf16)
    pmr = din("pmr", (1, T))                # padding mask row * -1e9

    w = {}
    for nme in ("mwq", "mwk", "mwv", "mwo", "cwq", "cwk", "cwv", "cwo"):
        w[nme] = din(nme, (128, DC, D))
    w["w1"] = din("w1", (128, DC, DFF))
    w["w2"] = din("w2", (128, HC, D))

    bq_m = din("bq_m", (128, DC), f32)      # m_bq/8 feature-major
    bk_m = din("bk_m", (128, DC), f32)
    bq_c = din("bq_c", (128, DC), f32)
    bk_c = din("bk_c", (128, DC), f32)
    b1 = din("b1", (128, HC), f32)
    bv_m = din("bv_m", (1, D))              # row biases (ones-matmul rhs)
    bo_m = din("bo_m", (1, D))
    bv_c = din("bv_c", (1, D))
    bo_c = din("bo_c", (1, D))
    b2 = din("b2", (1, D))
    lng = din("lng", (128, 3, D), f32)      # ln{1,2,3}_g pre-broadcast
    lnb = din("lnb", (128, 3, D), f32)

    out1_d = nc.dram_tensor("out1_d", [128, RC, D], f32)
    out2_d = nc.dram_tensor("out2_d", [128, RC, D], f32)
    rb_m = nc.dram_tensor("rb_m", [H * R], f32)   # per-head 1/rowsum scratch
    rb_c = nc.dram_tensor("rb_c", [H * R], f32)
    att_out_dt = bf16 if KDT == "bf16" else f32
    m_attn_o = nc.dram_tensor("m_attn_o", [H, R, S], att_out_dt, kind="ExternalOutput")
    c_attn_o = nc.dram_tensor("c_attn_o", [H, R, T], att_out_dt, kind="ExternalOutput")
    out3_o = nc.dram_tensor("out3_o", [R, D], f32, kind="ExternalOutput")

    ident_f = nc.inline_tensor(np.eye(128, dtype=np.float32), name="ident_f")
    ident_b = nc.inline_tensor(np.eye(128, dtype=ml_dtypes.bfloat16),
                               name="ident_b")
    if KDT == "bf16":
        ident_m = ident_b
        ones1 = nc.inline_tensor(np.ones((1, 128), dtype=ml_dtypes.bfloat16),
                                 name="ones1")
    else:
        ident_m = nc.inline_tensor(np.eye(128, dtype=np.float32),
                                   name="ident_r").bitcast(f32r)
        ones1 = nc.inline_tensor(np.ones((1, 128), dtype=np.float32),
                                 name="ones1").bitcast(f32r)

    with tile.TileContext(nc) as tc, ExitStack() as top:
        const = top.enter_context(tc.tile_pool(name="const", bufs=1))

        c_ident_f = const.tile([128, 128], f32, tag="ident_f")
        nc.sync.dma_start(out=c_ident_f, in_=ident_f[:, :])
        c_ident_b = const.tile([128, 128], bf16, tag="ident_b")
        nc.sync.dma_start(out=c_ident_b, in_=ident_b[:, :])
        c_ident_m = const.tile([128, 128], MDT, tag="ident_m")
        nc.sync.dma_start(out=c_ident_m, in_=ident_m[:, :])
        c_ones = const.tile([1, 128], MDT, tag="ones1")
        nc.sync.dma_start(out=c_ones, in_=ones1[:, :])
        c_eps = const.tile([128, 1], f32, tag="eps")
        nc.vector.memset(c_eps, EPS)

        def load_const(handle, shape, dt=f32):
            t = const.tile(list(shape), dt, tag=handle.name)
            nc.sync.dma_start(out=t,
                              in_=handle[tuple(slice(None) for _ in shape)])
            return t

        c_bq_m = load_const(bq_m, (128, DC))
        c_bk_m = load_const(bk_m, (128, DC))
        c_bq_c = load_const(bq_c, (128, DC))
        c_bk_c = load_const(bk_c, (128, DC))
        c_b1 = load_const(b1, (128, HC))
        c_bv_m = load_const(bv_m, (1, D), MDT)
        c_bo_m = load_const(bo_m, (1, D), MDT)
        c_bv_c = load_const(bv_c, (1, D), MDT)
        c_bo_c = load_const(bo_c, (1, D), MDT)
        c_b2 = load_const(b2, (1, D), MDT)
        c_pm = load_const(pmr, (1, T), MDT)

        # ------------------------ helpers ------------------------
        def fm_proj(w_dram, rhs_sb, ncols, out_sb, bias_sb,
                    func=AF.Identity, n_dout=DC, wtag="w", wbufs=2):
            """feature-major projection: out[:, oc, :] = act(sum_ic
            w[ic-chunk, oc-chunk].T @ rhs[:, ic, :] + bias[:, oc])."""
            with tc.tile_pool(name=wtag, bufs=wbufs) as wpool, \
                 tc.tile_pool(name=wtag + "p", bufs=2, space="PSUM") as pp:
                for hh in range((n_dout * 128) // 512):
                    wh = wpool.tile([128, DC, 512], MDT, tag=wtag)
                    nc.sync.dma_start(out=wh,
                                      in_=w_dram[:, :, hh * 512:(hh + 1) * 512])
                    for o in range(4):
                        oc = hh * 4 + o
                        ps = pp.tile([128, ncols], f32, tag=wtag + "p")
                        for kh in range(ncols // 512):
                            cs = slice(kh * 512, (kh + 1) * 512)
                            for ic in range(DC):
                                nc.tensor.matmul(
                                    ps[:, cs], wh[:, ic, o * 128:(o + 1) * 128],
                                    rhs_sb[:, ic, cs],
                                    start=(ic == 0), stop=(ic == DC - 1))
                        nc.scalar.activation(out_sb[:, oc, :], ps, func,
                                             bias=bias_sb[:, oc:oc + 1],
                                             scale=1.0)

        def v_proj(w_dram, lhs_sb, out_sb, bias_row, wtag, wbufs=2):
            """row-major projection from feature-major lhs (x^T chunks)."""
            with tc.tile_pool(name=wtag, bufs=wbufs) as wpool, \
                 tc.tile_pool(name=wtag + "p", bufs=2, space="PSUM") as pp:
                for dh in range(2):
                    wh = wpool.tile([128, DC, 512], MDT, tag=wtag)
                    nc.sync.dma_start(out=wh,
                                      in_=w_dram[:, :, dh * 512:(dh + 1) * 512])
                    for kc in range(DC):
                        ps = pp.tile([128, 512], f32, tag=wtag + "p")
                        for ic in range(DC):
                            nc.tensor.matmul(
                                ps, lhs_sb[:, ic, kc * 128:(kc + 1) * 128],
                                wh[:, ic, :], start=(ic == 0), stop=False)
                        nc.tensor.matmul(ps, c_ones,
                                         bias_row[0:1, dh * 512:(dh + 1) * 512],
                                         start=False, stop=True)
                        nc.any.tensor_copy(out_sb[:, kc, dh * 512:(dh + 1) * 512],
                                           ps)

        def o_proj(w_dram, avt_sb, res_rm, out_rm_t, bias_row, wtag, wbufs=1):
            """row-major out-projection + residual add into out_rm_t (f32)."""
            with tc.tile_pool(name=wtag, bufs=wbufs) as wpool, \
                 tc.tile_pool(name=wtag + "p", bufs=2, space="PSUM") as pp:
                for dh in range(2):
                    wh = wpool.tile([128, DC, 512], MDT, tag=wtag)
                    nc.sync.dma_start(out=wh,
                                      in_=w_dram[:, :, dh * 512:(dh + 1) * 512])
                    for rc in range(RC):
                        ps = pp.tile([128, 512], f32, tag=wtag + "p")
                        for ic in range(DC):
                            nc.tensor.matmul(
                                ps, avt_sb[:, ic, rc * 128:(rc + 1) * 128],
                                wh[:, ic, :], start=(ic == 0), stop=False)
                        nc.tensor.matmul(ps, c_ones,
                                         bias_row[0:1, dh * 512:(dh + 1) * 512],
                                         start=False, stop=True)
                        nc.vector.tensor_tensor(
                            out=out_rm_t[:, rc, dh * 512:(dh + 1) * 512],
                            in0=ps, in1=res_rm[:, rc, dh * 512:(dh + 1) * 512],
                            op=OP.add)

        def layernorm(li, in_rm, out_ap_fn):
            """row-major LN over free dim D; out_ap_fn(rc) -> DRAM AP dest."""
            with tc.tile_pool(name="ln%d" % li, bufs=4) as sp, \
                 tc.tile_pool(name="lno%d" % li, bufs=2) as op, \
                 tc.tile_pool(name="lngb%d" % li, bufs=1) as gp:
                gt = gp.tile([128, D], f32, tag="g")
                nc.sync.dma_start(out=gt, in_=lng[:, li, :])
                bt = gp.tile([128, D], f32, tag="b")
                nc.sync.dma_start(out=bt, in_=lnb[:, li, :])
                for rc in range(RC):
                    st = sp.tile([128, 2, 6], f32, tag="st")
                    nc.vector.bn_stats(out=st[:, 0, :], in_=in_rm[:, rc, 0:512])
                    nc.vector.bn_stats(out=st[:, 1, :], in_=in_rm[:, rc, 512:1024])
                    mv = sp.tile([128, 2], f32, tag="mv")
                    nc.vector.bn_aggr(out=mv, in_=st)
                    rs = sp.tile([128, 1], f32, tag="rs")
                    nc.scalar.activation(rs, mv[:, 1:2], AF.Sqrt,
                                         bias=c_eps, scale=1.0)
                    nc.vector.reciprocal(rs, rs)
                    lo = op.tile([128, D], f32, tag="lo")
                    nc.vector.tensor_scalar(out=lo, in0=in_rm[:, rc, :],
                                            scalar1=mv[:, 0:1], scalar2=rs,
                                            op0=OP.subtract, op1=OP.mult)
                    nc.vector.tensor_tensor(out=lo, in0=lo, in1=gt, op=OP.mult)
                    nc.vector.tensor_tensor(out=lo, in0=lo, in1=bt, op=OP.add)
                    nc.sync.dma_start(out=out_ap_fn(rc), in_=lo)

        def transpose_rm_to_fm(in_rm, out_fm, tag):
            """[128,RC,D] row-major f32 -> [128,DC,R] feature-major MDT."""
            with tc.tile_pool(name=tag + "p", bufs=2, space="PSUM") as pp:
                for dc in range(DC):
                    pt = pp.tile([128, R], f32, tag=tag + "p")
                    for rc in range(RC):
                        nc.tensor.transpose(pt[:, rc * 128:(rc + 1) * 128],
                                            in_rm[:, rc, dc * 128:(dc + 1) * 128],
                                            c_ident_f)
                    nc.any.tensor_copy(out_fm[:, dc, :], pt)

        def attention(prefix, qt_sb, kt_sb, v_sb, skey, mb_dram, attn_out,
                      avt_sb, rb_dram):
            """512 q-rows x skey keys, 16 heads. Normalized probs -> attn_out;
            A@V (normalized at eviction via DRAM-bounced 1/rowsum broadcast)
            -> avt_sb feature-major."""
            KCH = skey // 128
            with tc.tile_pool(name=prefix + "ae", bufs=5) as ap, \
                 tc.tile_pool(name=prefix + "an", bufs=3) as anp, \
                 tc.tile_pool(name=prefix + "anT", bufs=2) as antp, \
                 tc.tile_pool(name=prefix + "s", bufs=8) as sp, \
                 tc.tile_pool(name=prefix + "rb", bufs=2) as rbp, \
                 tc.tile_pool(name=prefix + "rbc", bufs=2) as rbcp, \
                 tc.tile_pool(name=prefix + "pl", bufs=3, space="PSUM") as pl, \
                 tc.tile_pool(name=prefix + "pt", bufs=1, space="PSUM") as pt, \
                 tc.tile_pool(name=prefix + "pv", bufs=1, space="PSUM") as pav:
                mb_sb = None
                if mb_dram is not None:
                    mb_sb = ap.tile([128, RC, S], bf16, tag=prefix + "mb",
                                    name=prefix + "_mb", bufs=1)
                    nc.sync.dma_start(out=mb_sb, in_=mb_dram[:, :, :])
                for h in range(H):
                    bp = (h % 2) * 64
                    fc = h // 2
                    rb = rbp.tile([128, RC], f32, tag=prefix + "rb")
                    ae_tiles = []
                    for qt in range(RC):
                        ps = pl.tile([128, skey], f32, tag=prefix + "L")
                        for kh in range(skey // 512):
                            cs = slice(kh * 512, (kh + 1) * 512)
                            nc.tensor.matmul(
                                ps[:, cs],
                                qt_sb[bp:bp + 64, fc, qt * 128:(qt + 1) * 128],
                                kt_sb[bp:bp + 64, fc, cs],
                                start=True, stop=False)
                            if mb_sb is not None:
                                nc.tensor.matmul(ps[:, cs], c_ident_b,
                                                 mb_sb[:, qt, cs],
                                                 start=False, stop=True)
                            else:
                                nc.tensor.matmul(ps[:, cs], c_ones,
                                                 c_pm[0:1, cs],
                                                 start=False, stop=True)
                        ae = ap.tile([128, skey], ATT_DT, tag=prefix + "ae")
                        ssum = sp.tile([128, 1], f32, tag=prefix + "sm")
                        nc.scalar.activation(ae, ps, AF.Exp, accum_out=ssum)
                        nc.vector.reciprocal(rb[:, qt:qt + 1], ssum)
                        an = anp.tile([128, skey], ATT_DT, tag=prefix + "an")
                        nc.vector.tensor_scalar(out=an, in0=ae,
                                                scalar1=rb[:, qt:qt + 1],
                                                scalar2=None, op0=OP.mult)
                        nc.sync.dma_start(
                            out=attn_out[h, qt * 128:(qt + 1) * 128, :],
                            in_=(an if KDT == "bf16" else an.bitcast(f32)))
                        ae_tiles.append(ae)
                    # 1/rowsum -> DRAM (transposed) -> [64, R] broadcast
                    nc.sync.dma_start(
                        out=bass.AP(tensor=rb_dram, offset=h * R,
                                    ap=[[1, 128], [128, RC]]),
                        in_=rb)
                    rbc = rbcp.tile([64, R], f32, tag=prefix + "rbc")
                    nc.sync.dma_start(
                        out=rbc,
                        in_=bass.AP(tensor=rb_dram, offset=h * R,
                                    ap=[[0, 64], [1, R]]))
                    anT = antp.tile([128, KCH, R], ATT_DT, tag=prefix + "anT")
                    for kc in range(KCH):
                        pst = pt.tile([128, R], ATT_DT, tag=prefix + "T")
                        for qt in range(RC):
                            nc.tensor.transpose(
                                pst[:, qt * 128:(qt + 1) * 128],
                                ae_tiles[qt][:, kc * 128:(kc + 1) * 128],
                                c_ident_m)
                        nc.any.tensor_copy(anT[:, kc, :], pst)
                    pa = pav.tile([64, R], f32, tag=prefix + "av")
                    for kc in range(KCH):
                        nc.tensor.matmul(pa, v_sb[:, kc, h * 64:(h + 1) * 64],
                                         anT[:, kc, :],
                                         start=(kc == 0), stop=(kc == KCH - 1))
                    nc.vector.tensor_tensor(out=avt_sb[bp:bp + 64, fc, :],
                                            in0=pa, in1=rbc, op=OP.mult)

        # ============ phases (strict LIFO pool nesting) ============
        pers = top.enter_context(tc.tile_pool(name="pers", bufs=1))

        # ---- phase 1+2: self attention ----
        with tc.tile_pool(name="sa", bufs=1) as sa:
            kt_sb = sa.tile([128, DC, S], MDT, tag="kt")
            qt_t = sa.tile([128, DC, R], MDT, tag="qt")
            v_sb = sa.tile([128, DC, D], MDT, tag="v")
            with tc.tile_pool(name="xp1", bufs=1) as xp1:
                xt_sb = xp1.tile([128, DC, S], MDT, tag="xt")
                nc.sync.dma_start(out=xt_sb, in_=xt[:, :, :])
                fm_proj(w["mwk"], xt_sb, S, kt_sb, c_bk_m, wtag="wk", wbufs=2)
                v_proj(w["mwv"], xt_sb, v_sb, c_bv_m, "wv", wbufs=2)
            with tc.tile_pool(name="xp2", bufs=1) as xp2:
                xtr_sb = xp2.tile([128, DC, R], MDT, tag="xtr")
                nc.sync.dma_start(out=xtr_sb, in_=xtr[:, :, :])
                fm_proj(w["mwq"], xtr_sb, R, qt_t, c_bq_m, wtag="wq", wbufs=2)
            avt_m = pers.tile([128, DC, R], MDT, tag="avt", name="avt_m")
            if KPH >= 2:
                attention("sa", qt_t, kt_sb, v_sb, S, mb, m_attn_o, avt_m, rb_m)

        # ---- phases 3+4 (overlapped): self out-proj + LN1 | cross proj ----
        if KPH >= 3:
            with tc.tile_pool(name="sc", bufs=1) as sc:
                kt_c = sc.tile([128, DC, T], MDT, tag="ktc")
                qt_c = sc.tile([128, DC, R], MDT, tag="qtc")
                v_c = sc.tile([128, DC, D], MDT, tag="vc")
                with tc.tile_pool(name="ep", bufs=1) as ep:
                    enct_sb = ep.tile([128, DC, T], MDT, tag="enct")
                    nc.sync.dma_start(out=enct_sb, in_=enct[:, :, :])
                    with tc.tile_pool(name="m1p", bufs=1) as m1p:
                        c_xrow = m1p.tile([128, RC, D], f32, tag="xrow")
                        nc.sync.dma_start(out=c_xrow, in_=xrow[:, :, :])
                        m1 = m1p.tile([128, RC, D], f32, tag="m1")
                        o_proj(w["mwo"], avt_m, c_xrow, m1, c_bo_m, "wo")
                        layernorm(0, m1, lambda rc: out1_d[:, rc, :])
                    if KPH >= 4:
                        fm_proj(w["cwk"], enct_sb, T, kt_c, c_bk_c,
                                wtag="wkc", wbufs=1)
                        v_proj(w["cwv"], enct_sb, v_c, c_bv_c, "wvc", wbufs=1)
                if KPH >= 4:
                    with tc.tile_pool(name="o1tp", bufs=1) as o1tp:
                        o1l = o1tp.tile([128, RC, D], f32, tag="o1l")
                        nc.sync.dma_start(out=o1l, in_=out1_d[:, :, :])
                        o1t = o1tp.tile([128, DC, R], MDT, tag="out1t")
                        transpose_rm_to_fm(o1l, o1t, "t1")
                        fm_proj(w["cwq"], o1t, R, qt_c, c_bq_c,
                                wtag="wqc", wbufs=2)
                avt_c = pers.tile([128, DC, R], MDT, tag="avt", name="avt_c")
                if KPH >= 5:
                    attention("ca", qt_c, kt_c, v_c, T, None, c_attn_o,
                              avt_c, rb_c)

        # ---- phases 6+7 (overlapped): cross out-proj + LN2 | FFN ----
        if KPH >= 6:
            with tc.tile_pool(name="ffp", bufs=1) as ffp:
                o2l = ffp.tile([128, RC, D], f32, tag="o2l")
                h1t = ffp.tile([128, HC, R], MDT, tag="h1t")
                m3 = ffp.tile([128, RC, D], f32, tag="m3")
                with tc.tile_pool(name="m2p", bufs=1) as m2p:
                    o1l2 = m2p.tile([128, RC, D], f32, tag="o1l2")
                    nc.sync.dma_start(out=o1l2, in_=out1_d[:, :, :])
                    m2 = m2p.tile([128, RC, D], f32, tag="m2")
                    o_proj(w["cwo"], avt_c, o1l2, m2, c_bo_c, "woc")
                    layernorm(1, m2, lambda rc: out2_d[:, rc, :])
                if KPH >= 7:
                    nc.sync.dma_start(out=o2l, in_=out2_d[:, :, :])
                    with tc.tile_pool(name="o2tp", bufs=1) as o2tp:
                        out2t = o2tp.tile([128, DC, R], MDT, tag="out2t")
                        transpose_rm_to_fm(o2l, out2t, "t2")
                        fm_proj(w["w1"], out2t, R, h1t, c_b1, func=AF.Relu,
                                n_dout=HC, wtag="w1", wbufs=2)
                    with tc.tile_pool(name="w2p", bufs=2) as w2p, \
                         tc.tile_pool(name="pf", bufs=1, space="PSUM") as pf:
                        for dh in range(2):
                            pstiles = [pf.tile([128, 512], f32, tag="pf%d" % rc,
                                               name="pf%d_%d" % (dh, rc))
                                       for rc in range(RC)]
                            for wc in range(4):
                                w2c = w2p.tile([128, 8, 512], MDT, tag="w2c")
                                nc.sync.dma_start(
                                    out=w2c,
                                    in_=w["w2"][:, wc * 8:(wc + 1) * 8,
                                                dh * 512:(dh + 1) * 512])
                                for rc in range(RC):
                                    ps = pstiles[rc]
                                    for i in range(8):
                                        hc = wc * 8 + i
                                        nc.tensor.matmul(
                                            ps,
                                            h1t[:, hc, rc * 128:(rc + 1) * 128],
                                            w2c[:, i, :],
                                            start=(hc == 0), stop=False)
                                    if wc == 3:
                                        nc.tensor.matmul(
                                            ps, c_ones,
                                            c_b2[0:1, dh * 512:(dh + 1) * 512],
                                            start=False, stop=True)
                                        nc.vector.tensor_tensor(
                                            out=m3[:, rc, dh * 512:(dh + 1) * 512],
                                            in0=ps,
                                            in1=o2l[:, rc, dh * 512:(dh + 1) * 512],
                                            op=OP.add)
                    layernorm(2, m3,
                              lambda rc: out3_o[rc * 128:(rc + 1) * 128, :])

    nc.finalize()
    return nc


def _fm(a):
    """[din, dout] -> [128, din/128, dout] (partition = din within chunk)"""
    din_, dout_ = a.shape
    return np.ascontiguousarray(
        a.reshape(din_ // 128, 128, dout_).transpose(1, 0, 2))


def _rm(a):
    rows, d_ = a.shape
    return np.ascontiguousarray(a.reshape(rows // 128, 128, d_).transpose(1, 0, 2))


def _prep_inputs(inputs):
    mdt = ml_dtypes.bfloat16 if KDT == "bf16" else np.float32
    f = lambda k: np.asarray(inputs[k], dtype=np.float32)
    x = f('x')
    enc = f('enc_output')
    lam = f('look_ahead_mask')
    pm = f('padding_mask')
    shared = {
        'mwq': _fm(f('m_wq') / 8.0).astype(mdt), 'mwk': _fm(f('m_wk')).astype(mdt),
        'mwv': _fm(f('m_wv')).astype(mdt), 'mwo': _fm(f('m_wo')).astype(mdt),
        'cwq': _fm(f('c_wq') / 8.0).astype(mdt), 'cwk': _fm(f('c_wk')).astype(mdt),
        'cwv': _fm(f('c_wv')).astype(mdt), 'cwo': _fm(f('c_wo')).astype(mdt),
        'w1': _fm(f('ffn_w1')).astype(mdt), 'w2': _fm(f('ffn_w2')).astype(mdt),
        'bq_m': np.ascontiguousarray((f('m_bq') / 8.0).reshape(DC, 128).T),
        'bk_m': np.ascontiguousarray(f('m_bk').reshape(DC, 128).T),
        'bq_c': np.ascontiguousarray((f('c_bq') / 8.0).reshape(DC, 128).T),
        'bk_c': np.ascontiguousarray(f('c_bk').reshape(DC, 128).T),
        'b1': np.ascontiguousarray(f('ffn_b1').reshape(HC, 128).T),
        'bv_m': f('m_bv').reshape(1, D).astype(mdt),
        'bo_m': f('m_bo').reshape(1, D).astype(mdt),
        'bv_c': f('c_bv').reshape(1, D).astype(mdt),
        'bo_c': f('c_bo').reshape(1, D).astype(mdt),
        'b2': f('ffn_b2').reshape(1, D).astype(mdt),
        'lng': np.ascontiguousarray(np.broadcast_to(
            np.stack([f('ln1_g'), f('ln2_g'), f('ln3_g')], 0)[None],
            (128, 3, D))),
        'lnb': np.ascontiguousarray(np.broadcast_to(
            np.stack([f('ln1_b'), f('ln2_b'), f('ln3_b')], 0)[None],
            (128, 3, D))),
    }
    in_maps = []
    for c in range(NC):
        b = c // 2
        off = (c % 2) * R
        m = dict(shared)
        m['xt'] = _fm(np.ascontiguousarray(x[b].T)).astype(mdt)
        m['xtr'] = _fm(np.ascontiguousarray(x[b, off:off + R].T)).astype(mdt)
        m['xrow'] = _rm(x[b, off:off + R])
        m['enct'] = _fm(np.ascontiguousarray(enc[b].T)).astype(mdt)
        mbr = (lam[0, 0, off:off + R, :] * np.float32(-1e9)).astype(
            ml_dtypes.bfloat16)
        m['mb'] = _rm(mbr)
        m['pmr'] = np.ascontiguousarray(
            (pm[b, 0, 0, :] * np.float32(-1e9)).reshape(1, T)).astype(mdt)
        in_maps.append(m)
    return in_maps


def _run(inputs, trace=False):
    if 'nc' not in _cache:
        _cache['nc'] = _build()
    nc = _cache['nc']
    in_maps = _prep_inputs(inputs)
    res = run_bass_kernel_spmd(nc, in_maps, list(range(NC)), trace=trace)
    out3 = np.zeros((B, S, D), np.float32)
    m_attn = np.zeros((B, H, S, S), np.float32)
    c_attn = np.zeros((B, H, S, T), np.float32)
    for c in range(NC):
        b = c // 2
        off = (c % 2) * R
        r = res.results[c]
        out3[b, off:off + R] = r['out3_o']
        m_attn[b, :, off:off + R, :] = np.asarray(r['m_attn_o'], np.float32)
        c_attn[b, :, off:off + R, :] = np.asarray(r['c_attn_o'], np.float32)
    return (out3, m_attn, c_attn), res


def kernel(**inputs):
    outs, _ = _run(inputs, trace=False)
    return outs


# revision 12
# speedup vs baseline: 1.0978x; 1.0978x over previous
"""Trainium2 Bass kernel for nn_DecoderLayer (B=4,S=T=1024,D=1024,H=16,DFF=4096).

Sharding: row-parallel (sequence-parallel over query rows). Core c owns 512
query rows = half of batch b=c//2 (row offset (c%2)*512). Each core recomputes
K/V for its whole batch -> zero collectives, perfectly uniform SPMD program.
Matmuls in float32r by default (full PE rate, ~1.6e-4 rel err; KDT=bf16 env
switches to bf16). Softmax: exp on ScalarE with fused row-sum (accum_out);
causal mask folded into the PE via an identity-matmul of a host-premultiplied
(-1e9) mask. The PE-side A@V chain consumes UNNORMALIZED exp tiles
(PE-transposed); normalization happens at the A@V eviction via a per-head
1/rowsum broadcast bounced through DRAM. The [q,k]-layout probs are
normalized on DVE and DMA'd out as m_attn/c_attn. LayerNorms row-major via
bn_stats/bn_aggr; out1/out2 spill to DRAM scratch to fit SBUF.
"""
import os
from contextlib import ExitStack

import numpy as np
import ml_dtypes

import concourse.bass as bass
from concourse import bacc
import concourse.mybir as mybir
import concourse.tile as tile
from concourse.bass_utils import run_bass_kernel_spmd

f32 = mybir.dt.float32
f32r = mybir.dt.float32r
bf16 = mybir.dt.bfloat16
AF = mybir.ActivationFunctionType
OP = mybir.AluOpType

B, S, T, D, H, DFF = 4, 1024, 1024, 1024, 16, 4096
DEPTH = D // H          # 64
NC = 8                  # cores
R = (B * S) // NC       # 512 rows per core
RC = R // 128           # 4 row chunks
DC = D // 128           # 8 feature chunks
HC = DFF // 128         # 32 hidden chunks
EPS = 1e-3

KDT = os.environ.get("KDT", "f32r")

_cache = {}


def _build():
    KPH = int(os.environ.get("KPHASES", "7"))
    MDT = bf16 if KDT == "bf16" else f32r
    ATT_DT = bf16 if KDT == "bf16" else f32r
    nc = bacc.Bacc(None, target_bir_lowering=False)

    def din(name, shape, dt=None):
        if dt is None:
            dt = MDT
        return nc.dram_tensor(name, list(shape), dt, kind="ExternalInput")

    xt = din("xt", (128, DC, S))            # x_b^T feature-major chunks
    xtr = din("xtr", (128, DC, R))          # x_rows^T
    xrow = din("xrow", (128, RC, D), f32)   # x rows row-major (residual 1)
    enct = din("enct", (128, DC, T))        # enc_b^T
    mb = din("mb", (128, RC, S), bf16)      # causal mask rows * -1e9 (bf16)
    pmr = din("pmr", (1, T))                # padding mask row * -1e9

    w = {}
    for nme in ("mwq", "mwk", "mwv", "mwo", "cwq", "cwk", "cwv", "cwo"):
        w[nme] = din(nme, (128, DC, D))
    w["w1"] = din("w1", (128, DC, DFF))
    w["w2"] = din("w2", (128, HC, D))

    bq_m = din("bq_m", (128, DC), f32)      # m_bq/8 feature-major
    bk_m = din("bk_m", (128, DC), f32)
    bq_c = din("bq_c", (128, DC), f32)
    bk_c = din("bk_c", (128, DC), f32)
    b1 = din("b1", (128, HC), f32)
    bv_m = din("bv_m", (1, D))              # row biases (ones-matmul rhs)
    bo_m = din("bo_m", (1, D))
    bv_c = din("bv_c", (1, D))
    bo_c = din("bo_c", (1, D))
    b2 = din("b2", (1, D))
    lng = din("lng", (128, 3, D), f32)      # ln{1,2,3}_g pre-broadcast
    lnb = din("lnb", (128, 3, D), f32)

    out1_d = nc.dram_tensor("out1_d", [128, RC, D], f32)
    out2_d = nc.dram_tensor("out2_d", [128, RC, D], f32)
    rb_m = nc.dram_tensor("rb_m", [H * R], f32)   # per-head 1/rowsum scratch
    rb_c = nc.dram_tensor("rb_c", [H * R], f32)
    att_out_dt = bf16 if KDT == "bf16" else f32
    m_attn_o = nc.dram_tensor("m_attn_o", [H, R, S], att_out_dt, kind="ExternalOutput")
    c_attn_o = nc.dram_tensor("c_attn_o", [H, R, T], att_out_dt, kind="ExternalOutput")
    out3_o = nc.dram_tensor("out3_o", [R, D], f32, kind="ExternalOutput")

    ident_f = nc.inline_tensor(np.eye(128, dtype=np.float32), name="ident_f")
    ident_b = nc.inline_tensor(np.eye(128, dtype=ml_dtypes.bfloat16),
                               name="ident_b")
    if KDT == "bf16":
        ident_m = ident_b
        ones1 = nc.inline_tensor(np.ones((1, 128), dtype=ml_dtypes.bfloat16),
                                 name="ones1")
    else:
        ident_m = nc.inline_tensor(np.eye(128, dtype=np.float32),
                                   name="ident_r").bitcast(f32r)
        ones1 = nc.inline_tensor(np.ones((1, 128), dtype=np.float32),
                                 name="ones1").bitcast(f32r)

    with tile.TileContext(nc) as tc, ExitStack() as top:
        const = top.enter_context(tc.tile_pool(name="const", bufs=1))

        c_ident_f = const.tile([128, 128], f32, tag="ident_f")
        nc.sync.dma_start(out=c_ident_f, in_=ident_f[:, :])
        c_ident_b = const.tile([128, 128], bf16, tag="ident_b")
        nc.sync.dma_start(out=c_ident_b, in_=ident_b[:, :])
        c_ident_m = const.tile([128, 128], MDT, tag="ident_m")
        nc.sync.dma_start(out=c_ident_m, in_=ident_m[:, :])
        c_ones = const.tile([1, 128], MDT, tag="ones1")
        nc.sync.dma_start(out=c_ones, in_=ones1[:, :])
        c_eps = const.tile([128, 1], f32, tag="eps")
        nc.vector.memset(c_eps, EPS)

        def load_const(handle, shape, dt=f32):
            t = const.tile(list(shape), dt, tag=handle.name)
            nc.sync.dma_start(out=t,
                              in_=handle[tuple(slice(None) for _ in shape)])
            return t

        c_bq_m = load_const(bq_m, (128, DC))
        c_bk_m = load_const(bk_m, (128, DC))
        c_bq_c = load_const(bq_c, (128, DC))
        c_bk_c = load_const(bk_c, (128, DC))
        c_b1 = load_const(b1, (128, HC))
        c_bv_m = load_const(bv_m, (1, D), MDT)
        c_bo_m = load_const(bo_m, (1, D), MDT)
        c_bv_c = load_const(bv_c, (1, D), MDT)
        c_bo_c = load_const(bo_c, (1, D), MDT)
        c_b2 = load_const(b2, (1, D), MDT)
        c_pm = load_const(pmr, (1, T), MDT)

        # ------------------------ helpers ------------------------
        def fm_proj(w_dram, rhs_sb, ncols, out_sb, bias_sb,
                    func=AF.Identity, n_dout=DC, wtag="w", wbufs=2):
            """feature-major projection: out[:, oc, :] = act(sum_ic
            w[ic-chunk, oc-chunk].T @ rhs[:, ic, :] + bias[:, oc])."""
            with tc.tile_pool(name=wtag, bufs=wbufs) as wpool, \
                 tc.tile_pool(name=wtag + "p", bufs=2, space="PSUM") as pp:
                for hh in range((n_dout * 128) // 512):
                    wh = wpool.tile([128, DC, 512], MDT, tag=wtag)
                    nc.sync.dma_start(out=wh,
                                      in_=w_dram[:, :, hh * 512:(hh + 1) * 512])
                    for o in range(4):
                        oc = hh * 4 + o
                        ps = pp.tile([128, ncols], f32, tag=wtag + "p")
                        for kh in range(ncols // 512):
                            cs = slice(kh * 512, (kh + 1) * 512)
                            for ic in range(DC):
                                nc.tensor.matmul(
                                    ps[:, cs], wh[:, ic, o * 128:(o + 1) * 128],
                                    rhs_sb[:, ic, cs],
                                    start=(ic == 0), stop=(ic == DC - 1))
                        nc.scalar.activation(out_sb[:, oc, :], ps, func,
                                             bias=bias_sb[:, oc:oc + 1],
                                             scale=1.0)

        def v_proj(w_dram, lhs_sb, out_sb, bias_row, wtag, wbufs=2):
            """row-major projection from feature-major lhs (x^T chunks)."""
            with tc.tile_pool(name=wtag, bufs=wbufs) as wpool, \
                 tc.tile_pool(name=wtag + "p", bufs=2, space="PSUM") as pp:
                for dh in range(2):
                    wh = wpool.tile([128, DC, 512], MDT, tag=wtag)
                    nc.sync.dma_start(out=wh,
                                      in_=w_dram[:, :, dh * 512:(dh + 1) * 512])
                    for kc in range(DC):
                        ps = pp.tile([128, 512], f32, tag=wtag + "p")
                        for ic in range(DC):
                            nc.tensor.matmul(
                                ps, lhs_sb[:, ic, kc * 128:(kc + 1) * 128],
                                wh[:, ic, :], start=(ic == 0), stop=False)
                        nc.tensor.matmul(ps, c_ones,
                                         bias_row[0:1, dh * 512:(dh + 1) * 512],
                                         start=False, stop=True)
                        nc.any.tensor_copy(out_sb[:, kc, dh * 512:(dh + 1) * 512],
                                           ps)

        def o_proj(w_dram, avt_sb, res_rm, out_rm_t, bias_row, wtag, wbufs=2):
            """row-major out-projection + residual add into out_rm_t (f32)."""
            with tc.tile_pool(name=wtag, bufs=wbufs) as wpool, \
                 tc.tile_pool(name=wtag + "p", bufs=2, space="PSUM") as pp:
                for dh in range(2):
                    wh = wpool.tile([128, DC, 512], MDT, tag=wtag)
                    nc.sync.dma_start(out=wh,
                                      in_=w_dram[:, :, dh * 512:(dh + 1) * 512])
                    for rc in range(RC):
                        ps = pp.tile([128, 512], f32, tag=wtag + "p")
                        for ic in range(DC):
                            nc.tensor.matmul(
                                ps, avt_sb[:, ic, rc * 128:(rc + 1) * 128],
                                wh[:, ic, :], start=(ic == 0), stop=False)
                        nc.tensor.matmul(ps, c_ones,
                                         bias_row[0:1, dh * 512:(dh + 1) * 512],
                                         start=False, stop=True)
                        nc.vector.tensor_tensor(
                            out=out_rm_t[:, rc, dh * 512:(dh + 1) * 512],
                            in0=ps, in1=res_rm[:, rc, dh * 512:(dh + 1) * 512],
                            op=OP.add)

        def layernorm(li, in_rm, out_ap_fn):
            """row-major LN over free dim D; out_ap_fn(rc) -> DRAM AP dest."""
            with tc.tile_pool(name="ln%d" % li, bufs=4) as sp, \
                 tc.tile_pool(name="lno%d" % li, bufs=2) as op, \
                 tc.tile_pool(name="lngb%d" % li, bufs=1) as gp:
                gt = gp.tile([128, D], f32, tag="g")
                nc.sync.dma_start(out=gt, in_=lng[:, li, :])
                bt = gp.tile([128, D], f32, tag="b")
                nc.sync.dma_start(out=bt, in_=lnb[:, li, :])
                for rc in range(RC):
                    st = sp.tile([128, 2, 6], f32, tag="st")
                    nc.vector.bn_stats(out=st[:, 0, :], in_=in_rm[:, rc, 0:512])
                    nc.vector.bn_stats(out=st[:, 1, :], in_=in_rm[:, rc, 512:1024])
                    mv = sp.tile([128, 2], f32, tag="mv")
                    nc.vector.bn_aggr(out=mv, in_=st)
                    rs = sp.tile([128, 1], f32, tag="rs")
                    nc.scalar.activation(rs, mv[:, 1:2], AF.Sqrt,
                                         bias=c_eps, scale=1.0)
                    nc.vector.reciprocal(rs, rs)
                    lo = op.tile([128, D], f32, tag="lo")
                    nc.vector.tensor_scalar(out=lo, in0=in_rm[:, rc, :],
                                            scalar1=mv[:, 0:1], scalar2=rs,
                                            op0=OP.subtract, op1=OP.mult)
                    nc.vector.tensor_tensor(out=lo, in0=lo, in1=gt, op=OP.mult)
                    nc.vector.tensor_tensor(out=lo, in0=lo, in1=bt, op=OP.add)
                    nc.sync.dma_start(out=out_ap_fn(rc), in_=lo)

        def transpose_rm_to_fm(in_rm, out_fm, tag):
            """[128,RC,D] row-major f32 -> [128,DC,R] feature-major MDT."""
            with tc.tile_pool(name=tag + "p", bufs=2, space="PSUM") as pp:
                for dc in range(DC):
                    pt = pp.tile([128, R], f32, tag=tag + "p")
                    for rc in range(RC):
                        nc.tensor.transpose(pt[:, rc * 128:(rc + 1) * 128],
                                            in_rm[:, rc, dc * 128:(dc + 1) * 128],
                                            c_ident_f)
                    nc.any.tensor_copy(out_fm[:, dc, :], pt)

        def attention(prefix, qt_sb, kt_sb, v_sb, skey, mb_dram, attn_out,
                      avt_sb, rb_dram):
            """512 q-rows x skey keys, 16 heads. Normalized probs -> attn_out;
            A@V (normalized at eviction via DRAM-bounced 1/rowsum broadcast)
            -> avt_sb feature-major."""
            KCH = skey // 128
            with tc.tile_pool(name=prefix + "ae", bufs=5) as ap, \
                 tc.tile_pool(name=prefix + "an", bufs=3) as anp, \
                 tc.tile_pool(name=prefix + "anT", bufs=2) as antp, \
                 tc.tile_pool(name=prefix + "s", bufs=8) as sp, \
                 tc.tile_pool(name=prefix + "rb", bufs=2) as rbp, \
                 tc.tile_pool(name=prefix + "rbc", bufs=2) as rbcp, \
                 tc.tile_pool(name=prefix + "pl", bufs=2, space="PSUM") as pl, \
                 tc.tile_pool(name=prefix + "pt", bufs=2, space="PSUM") as pt, \
                 tc.tile_pool(name=prefix + "pv", bufs=2, space="PSUM") as pav:
                mb_sb = None
                if mb_dram is not None:
                    mb_sb = ap.tile([128, RC, S], bf16, tag=prefix + "mb",
                                    name=prefix + "_mb", bufs=1)
                    nc.sync.dma_start(out=mb_sb, in_=mb_dram[:, :, :])
                for h in range(H):
                    bp = (h % 2) * 64
                    fc = h // 2
                    rb = rbp.tile([128, RC], f32, tag=prefix + "rb")
                    ae_tiles = []
                    for qt in range(RC):
                        ps = pl.tile([128, skey], f32, tag=prefix + "L")
                        for kh in range(skey // 512):
                            cs = slice(kh * 512, (kh + 1) * 512)
                            nc.tensor.matmul(
                                ps[:, cs],
                                qt_sb[bp:bp + 64, fc, qt * 128:(qt + 1) * 128],
                                kt_sb[bp:bp + 64, fc, cs],
                                start=True, stop=False)
                            if mb_sb is not None:
                                nc.tensor.matmul(ps[:, cs], c_ident_b,
                                                 mb_sb[:, qt, cs],
                                                 start=False, stop=True)
                            else:
                                nc.tensor.matmul(ps[:, cs], c_ones,
                                                 c_pm[0:1, cs],
                                                 start=False, stop=True)
                        ae = ap.tile([128, skey], ATT_DT, tag=prefix + "ae")
                        ssum = sp.tile([128, 1], f32, tag=prefix + "sm")
                        nc.scalar.activation(ae, ps, AF.Exp, accum_out=ssum)
                        nc.vector.reciprocal(rb[:, qt:qt + 1], ssum)
                        an = anp.tile([128, skey], ATT_DT, tag=prefix + "an")
                        nc.vector.tensor_scalar(out=an, in0=ae,
                                                scalar1=rb[:, qt:qt + 1],
                                                scalar2=None, op0=OP.mult)
                        nc.sync.dma_start(
                            out=attn_out[h, qt * 128:(qt + 1) * 128, :],
                            in_=(an if KDT == "bf16" else an.bitcast(f32)))
                        ae_tiles.append(ae)
                    # 1/rowsum -> DRAM (transposed) -> [64, R] broadcast
                    nc.sync.dma_start(
                        out=bass.AP(tensor=rb_dram, offset=h * R,
                                    ap=[[1, 128], [128, RC]]),
                        in_=rb)
                    rbc = rbcp.tile([64, R], f32, tag=prefix + "rbc")
                    nc.sync.dma_start(
                        out=rbc,
                        in_=bass.AP(tensor=rb_dram, offset=h * R,
                                    ap=[[0, 64], [1, R]]))
                    anT = antp.tile([128, KCH, R], ATT_DT, tag=prefix + "anT")
                    for kc in range(KCH):
                        pst = pt.tile([128, R], ATT_DT, tag=prefix + "T")
                        for qt in range(RC):
                            nc.tensor.transpose(
                                pst[:, qt * 128:(qt + 1) * 128],
                                ae_tiles[qt][:, kc * 128:(kc + 1) * 128],
                                c_ident_m)
                        nc.any.tensor_copy(anT[:, kc, :], pst)
                    pa = pav.tile([64, R], f32, tag=prefix + "av")
                    for kc in range(KCH):
                        nc.tensor.matmul(pa, v_sb[:, kc, h * 64:(h + 1) * 64],
                                         anT[:, kc, :],
                                         start=(kc == 0), stop=(kc == KCH - 1))
                    nc.vector.tensor_tensor(out=avt_sb[bp:bp + 64, fc, :],
                                            in0=pa, in1=rbc, op=OP.mult)

        # ============ phases (strict LIFO pool nesting) ============
        pers = top.enter_context(tc.tile_pool(name="pers", bufs=1))

        # ---- phase 1+2: self attention ----
        with tc.tile_pool(name="sa", bufs=1) as sa:
            kt_sb = sa.tile([128, DC, S], MDT, tag="kt")
            qt_t = sa.tile([128, DC, R], MDT, tag="qt")
            v_sb = sa.tile([128, DC, D], MDT, tag="v")
            with tc.tile_pool(name="xp1", bufs=1) as xp1:
                xt_sb = xp1.tile([128, DC, S], MDT, tag="xt")
                nc.sync.dma_start(out=xt_sb, in_=xt[:, :, :])
                fm_proj(w["mwk"], xt_sb, S, kt_sb, c_bk_m, wtag="wk", wbufs=2)
                v_proj(w["mwv"], xt_sb, v_sb, c_bv_m, "wv", wbufs=2)
            with tc.tile_pool(name="xp2", bufs=1) as xp2:
                xtr_sb = xp2.tile([128, DC, R], MDT, tag="xtr")
                nc.sync.dma_start(out=xtr_sb, in_=xtr[:, :, :])
                fm_proj(w["mwq"], xtr_sb, R, qt_t, c_bq_m, wtag="wq", wbufs=2)
            avt_m = pers.tile([128, DC, R], MDT, tag="avt", name="avt_m")
            if KPH >= 2:
                attention("sa", qt_t, kt_sb, v_sb, S, mb, m_attn_o, avt_m, rb_m)

        # ---- phases 3+4 (overlapped): self out-proj + LN1 | cross proj ----
        if KPH >= 3:
            with tc.tile_pool(name="sc", bufs=1) as sc:
                kt_c = sc.tile([128, DC, T], MDT, tag="ktc")
                qt_c = sc.tile([128, DC, R], MDT, tag="qtc")
                v_c = sc.tile([128, DC, D], MDT, tag="vc")
                with tc.tile_pool(name="ep", bufs=1) as ep:
                    enct_sb = ep.tile([128, DC, T], MDT, tag="enct")
                    nc.sync.dma_start(out=enct_sb, in_=enct[:, :, :])
                    with tc.tile_pool(name="m1p", bufs=1) as m1p:
                        c_xrow = m1p.tile([128, RC, D], f32, tag="xrow")
                        nc.sync.dma_start(out=c_xrow, in_=xrow[:, :, :])
                        m1 = m1p.tile([128, RC, D], f32, tag="m1")
                        o_proj(w["mwo"], avt_m, c_xrow, m1, c_bo_m, "wo", wbufs=1)
                        layernorm(0, m1, lambda rc: out1_d[:, rc, :])
                    if KPH >= 4:
                        fm_proj(w["cwk"], enct_sb, T, kt_c, c_bk_c,
                                wtag="wkc", wbufs=2)
                        v_proj(w["cwv"], enct_sb, v_c, c_bv_c, "wvc", wbufs=2)
                if KPH >= 4:
                    with tc.tile_pool(name="o1tp", bufs=1) as o1tp:
                        o1l = o1tp.tile([128, RC, D], f32, tag="o1l")
                        nc.sync.dma_start(out=o1l, in_=out1_d[:, :, :])
                        o1t = o1tp.tile([128, DC, R], MDT, tag="out1t")
                        transpose_rm_to_fm(o1l, o1t, "t1")
                        fm_proj(w["cwq"], o1t, R, qt_c, c_bq_c,
                                wtag="wqc", wbufs=2)
                avt_c = pers.tile([128, DC, R], MDT, tag="avt", name="avt_c")
                if KPH >= 5:
                    attention("ca", qt_c, kt_c, v_c, T, None, c_attn_o,
                              avt_c, rb_c)

        # ---- phases 6+7 (overlapped): cross out-proj + LN2 | FFN ----
        if KPH >= 6:
            with tc.tile_pool(name="ffp", bufs=1) as ffp:
                o2l = ffp.tile([128, RC, D], f32, tag="o2l")
                h1t = ffp.tile([128, HC, R], MDT, tag="h1t")
                m3 = ffp.tile([128, RC, D], f32, tag="m3")
                with tc.tile_pool(name="m2p", bufs=1) as m2p:
                    o1l2 = m2p.tile([128, RC, D], f32, tag="o1l2")
                    nc.sync.dma_start(out=o1l2, in_=out1_d[:, :, :])
                    m2 = m2p.tile([128, RC, D], f32, tag="m2")
                    o_proj(w["cwo"], avt_c, o1l2, m2, c_bo_c, "woc")
                    layernorm(1, m2, lambda rc: out2_d[:, rc, :])
                if KPH >= 7:
                    nc.sync.dma_start(out=o2l, in_=out2_d[:, :, :])
                    with tc.tile_pool(name="o2tp", bufs=1) as o2tp:
                        out2t = o2tp.tile([128, DC, R], MDT, tag="out2t")
                        transpose_rm_to_fm(o2l, out2t, "t2")
                        fm_proj(w["w1"], out2t, R, h1t, c_b1, func=AF.Relu,
                                n_dout=HC, wtag="w1", wbufs=2)
                    with tc.tile_pool(name="w2p", bufs=2) as w2p, \
                         tc.tile_pool(name="pf", bufs=1, space="PSUM") as pf:
                        for dh in range(2):
                            pstiles = [pf.tile([128, 512], f32, tag="pf%d" % rc,
                                               name="pf%d_%d" % (dh, rc))
                                       for rc in range(RC)]
                            for wc in range(4):
                                w2c = w2p.tile([128, 8, 512], MDT, tag="w2c")
                                nc.sync.dma_start(
                                    out=w2c,
                                    in_=w["w2"][:, wc * 8:(wc + 1) * 8,
                                                dh * 512:(dh + 1) * 512])
                                for rc in range(RC):
                                    ps = pstiles[rc]
                                    for i in range(8):
                                        hc = wc * 8 + i
                                        nc.tensor.matmul(
                                            ps,
                                            h1t[:, hc, rc * 128:(rc + 1) * 128],
                                            w2c[:, i, :],
                                            start=(hc == 0), stop=False)
                                    if wc == 3:
                                        nc.tensor.matmul(
                                            ps, c_ones,
                                            c_b2[0:1, dh * 512:(dh + 1) * 512],
                                            start=False, stop=True)
                                        nc.vector.tensor_tensor(
                                            out=m3[:, rc, dh * 512:(dh + 1) * 512],
                                            in0=ps,
                                            in1=o2l[:, rc, dh * 512:(dh + 1) * 512],
                                            op=OP.add)
                    layernorm(2, m3,
                              lambda rc: out3_o[rc * 128:(rc + 1) * 128, :])

    nc.finalize()
    return nc


def _fm(a):
    """[din, dout] -> [128, din/128, dout] (partition = din within chunk)"""
    din_, dout_ = a.shape
    return np.ascontiguousarray(
        a.reshape(din_ // 128, 128, dout_).transpose(1, 0, 2))


def _rm(a):
    rows, d_ = a.shape
    return np.ascontiguousarray(a.reshape(rows // 128, 128, d_).transpose(1, 0, 2))


def _prep_inputs(inputs):
    mdt = ml_dtypes.bfloat16 if KDT == "bf16" else np.float32
    f = lambda k: np.asarray(inputs[k], dtype=np.float32)
    x = f('x')
    enc = f('enc_output')
    lam = f('look_ahead_mask')
    pm = f('padding_mask')
    shared = {
        'mwq': _fm(f('m_wq') / 8.0).astype(mdt), 'mwk': _fm(f('m_wk')).astype(mdt),
        'mwv': _fm(f('m_wv')).astype(mdt), 'mwo': _fm(f('m_wo')).astype(mdt),
        'cwq': _fm(f('c_wq') / 8.0).astype(mdt), 'cwk': _fm(f('c_wk')).astype(mdt),
        'cwv': _fm(f('c_wv')).astype(mdt), 'cwo': _fm(f('c_wo')).astype(mdt),
        'w1': _fm(f('ffn_w1')).astype(mdt), 'w2': _fm(f('ffn_w2')).astype(mdt),
        'bq_m': np.ascontiguousarray((f('m_bq') / 8.0).reshape(DC, 128).T),
        'bk_m': np.ascontiguousarray(f('m_bk').reshape(DC, 128).T),
        'bq_c': np.ascontiguousarray((f('c_bq') / 8.0).reshape(DC, 128).T),
        'bk_c': np.ascontiguousarray(f('c_bk').reshape(DC, 128).T),
        'b1': np.ascontiguousarray(f('ffn_b1').reshape(HC, 128).T),
        'bv_m': f('m_bv').reshape(1, D).astype(mdt),
        'bo_m': f('m_bo').reshape(1, D).astype(mdt),
        'bv_c': f('c_bv').reshape(1, D).astype(mdt),
        'bo_c': f('c_bo').reshape(1, D).astype(mdt),
        'b2': f('ffn_b2').reshape(1, D).astype(mdt),
        'lng': np.ascontiguousarray(np.broadcast_to(
            np.stack([f('ln1_g'), f('ln2_g'), f('ln3_g')], 0)[None],
            (128, 3, D))),
        'lnb': np.ascontiguousarray(np.broadcast_to(
            np.stack([f('ln1_b'), f('ln2_b'), f('ln3_b')], 0)[None],
            (128, 3, D))),
    }
    in_maps = []
    for c in range(NC):
        b = c // 2
        off = (c % 2) * R
        m = dict(shared)
        m['xt'] = _fm(np.ascontiguousarray(x[b].T)).astype(mdt)
        m['xtr'] = _fm(np.ascontiguousarray(x[b, off:off + R].T)).astype(mdt)
        m['xrow'] = _rm(x[b, off:off + R])
        m['enct'] = _fm(np.ascontiguousarray(enc[b].T)).astype(mdt)
        mbr = (lam[0, 0, off:off + R, :] * np.float32(-1e9)).astype(
            ml_dtypes.bfloat16)
        m['mb'] = _rm(mbr)
        m['pmr'] = np.ascontiguousarray(
            (pm[b, 0, 0, :] * np.float32(-1e9)).reshape(1, T)).astype(mdt)
        in_maps.append(m)
    return in_maps


def _run(inputs, trace=False):
    if 'nc' not in _cache:
        _cache['nc'] = _build()
    nc = _cache['nc']
    in_maps = _prep_inputs(inputs)
    res = run_bass_kernel_spmd(nc, in_maps, list(range(NC)), trace=trace)
    out3 = np.zeros((B, S, D), np.float32)
    m_attn = np.zeros((B, H, S, S), np.float32)
    c_attn = np.zeros((B, H, S, T), np.float32)
    for c in range(NC):
        b = c // 2
        off = (c % 2) * R
        r = res.results[c]
        out3[b, off:off + R] = r['out3_o']
        m_attn[b, :, off:off + R, :] = np.asarray(r['m_attn_o'], np.float32)
        c_attn[b, :, off:off + R, :] = np.asarray(r['c_attn_o'], np.float32)
    return (out3, m_attn, c_attn), res


def kernel(**inputs):
    outs, _ = _run(inputs, trace=False)
    return outs


# revision 13
# speedup vs baseline: 1.1560x; 1.0531x over previous
"""Trainium2 Bass kernel for nn_DecoderLayer (B=4,S=T=1024,D=1024,H=16,DFF=4096).

Sharding: row-parallel (sequence-parallel over query rows). Core c owns 512
query rows = half of batch b=c//2 (row offset (c%2)*512). Each core recomputes
K/V for its whole batch -> zero collectives, perfectly uniform SPMD program.
Matmuls in float32r by default (full PE rate, ~1.6e-4 rel err; KDT=bf16 env
switches to bf16). Softmax: exp on ScalarE with fused row-sum (accum_out);
causal mask folded into the PE via an identity-matmul of a host-premultiplied
(-1e9) mask. The PE-side A@V chain consumes UNNORMALIZED exp tiles
(PE-transposed); normalization happens at the A@V eviction via a per-head
1/rowsum broadcast bounced through DRAM. The [q,k]-layout probs are
normalized on DVE and DMA'd out as m_attn/c_attn. LayerNorms row-major via
bn_stats/bn_aggr; out1/out2 spill to DRAM scratch to fit SBUF.
"""
import os
from contextlib import ExitStack

import numpy as np
import ml_dtypes

import concourse.bass as bass
from concourse import bacc
import concourse.mybir as mybir
import concourse.tile as tile
from concourse.bass_utils import run_bass_kernel_spmd

f32 = mybir.dt.float32
f32r = mybir.dt.float32r
bf16 = mybir.dt.bfloat16
AF = mybir.ActivationFunctionType
OP = mybir.AluOpType

B, S, T, D, H, DFF = 4, 1024, 1024, 1024, 16, 4096
DEPTH = D // H          # 64
NC = 8                  # cores
R = (B * S) // NC       # 512 rows per core
RC = R // 128           # 4 row chunks
DC = D // 128           # 8 feature chunks
HC = DFF // 128         # 32 hidden chunks
EPS = 1e-3

KDT = os.environ.get("KDT", "f32r")

_cache = {}


def _build():
    KPH = int(os.environ.get("KPHASES", "7"))
    MDT = bf16 if KDT == "bf16" else f32r
    ATT_DT = bf16 if KDT == "bf16" else f32r
    nc = bacc.Bacc(None, target_bir_lowering=False)

    def din(name, shape, dt=None):
        if dt is None:
            dt = MDT
        return nc.dram_tensor(name, list(shape), dt, kind="ExternalInput")

    xt = din("xt", (128, DC, S))            # x_b^T feature-major chunks
    xtr = din("xtr", (128, DC, R))          # x_rows^T
    xrow = din("xrow", (128, RC, D), f32)   # x rows row-major (residual 1)
    enct = din("enct", (128, DC, T))        # enc_b^T
    mb = din("mb", (128, RC, S), bf16)      # causal mask rows * -1e9 (bf16)
    pmr = din("pmr", (1, T))                # padding mask row * -1e9

    w = {}
    for nme in ("mwq", "mwk", "mwv", "mwo", "cwq", "cwk", "cwv", "cwo"):
        w[nme] = din(nme, (128, DC, D))
    w["w1"] = din("w1", (128, DC, DFF))
    w["w2"] = din("w2", (128, HC, D))

    bq_m = din("bq_m", (128, DC), f32)      # m_bq/8 feature-major
    bk_m = din("bk_m", (128, DC), f32)
    bq_c = din("bq_c", (128, DC), f32)
    bk_c = din("bk_c", (128, DC), f32)
    b1 = din("b1", (128, HC), f32)
    bv_m = din("bv_m", (1, D))              # row biases (ones-matmul rhs)
    bo_m = din("bo_m", (1, D))
    bv_c = din("bv_c", (1, D))
    bo_c = din("bo_c", (1, D))
    b2 = din("b2", (1, D))
    lng = din("lng", (128, 3, D), f32)      # ln{1,2,3}_g pre-broadcast
    lnb = din("lnb", (128, 3, D), f32)

    out1_d = nc.dram_tensor("out1_d", [128, RC, D], f32)
    out2_d = nc.dram_tensor("out2_d", [128, RC, D], f32)
    rb_m = nc.dram_tensor("rb_m", [H * R], f32)   # per-head 1/rowsum scratch
    rb_c = nc.dram_tensor("rb_c", [H * R], f32)
    att_out_dt = bf16 if KDT == "bf16" else f32
    m_attn_o = nc.dram_tensor("m_attn_o", [H, R, S], att_out_dt, kind="ExternalOutput")
    c_attn_o = nc.dram_tensor("c_attn_o", [H, R, T], att_out_dt, kind="ExternalOutput")
    out3_o = nc.dram_tensor("out3_o", [R, D], f32, kind="ExternalOutput")

    ident_f = nc.inline_tensor(np.eye(128, dtype=np.float32), name="ident_f")
    ident_b = nc.inline_tensor(np.eye(128, dtype=ml_dtypes.bfloat16),
                               name="ident_b")
    if KDT == "bf16":
        ident_m = ident_b
        ones1 = nc.inline_tensor(np.ones((1, 128), dtype=ml_dtypes.bfloat16),
                                 name="ones1")
    else:
        ident_m = nc.inline_tensor(np.eye(128, dtype=np.float32),
                                   name="ident_r").bitcast(f32r)
        ones1 = nc.inline_tensor(np.ones((1, 128), dtype=np.float32),
                                 name="ones1").bitcast(f32r)

    with tile.TileContext(nc) as tc, ExitStack() as top:
        const = top.enter_context(tc.tile_pool(name="const", bufs=1))

        c_ident_f = const.tile([128, 128], f32, tag="ident_f")
        nc.sync.dma_start(out=c_ident_f, in_=ident_f[:, :])
        c_ident_b = const.tile([128, 128], bf16, tag="ident_b")
        nc.sync.dma_start(out=c_ident_b, in_=ident_b[:, :])
        c_ident_m = const.tile([128, 128], MDT, tag="ident_m")
        nc.sync.dma_start(out=c_ident_m, in_=ident_m[:, :])
        c_ones = const.tile([1, 128], MDT, tag="ones1")
        nc.sync.dma_start(out=c_ones, in_=ones1[:, :])
        c_eps = const.tile([128, 1], f32, tag="eps")
        nc.vector.memset(c_eps, EPS)

        def load_const(handle, shape, dt=f32):
            t = const.tile(list(shape), dt, tag=handle.name)
            nc.sync.dma_start(out=t,
                              in_=handle[tuple(slice(None) for _ in shape)])
            return t

        c_bq_m = load_const(bq_m, (128, DC))
        c_bk_m = load_const(bk_m, (128, DC))
        c_bq_c = load_const(bq_c, (128, DC))
        c_bk_c = load_const(bk_c, (128, DC))
        c_b1 = load_const(b1, (128, HC))
        c_bv_m = load_const(bv_m, (1, D), MDT)
        c_bo_m = load_const(bo_m, (1, D), MDT)
        c_bv_c = load_const(bv_c, (1, D), MDT)
        c_bo_c = load_const(bo_c, (1, D), MDT)
        c_b2 = load_const(b2, (1, D), MDT)
        c_pm = load_const(pmr, (1, T), MDT)

        # ------------------------ helpers ------------------------
        def fm_proj(w_dram, rhs_sb, ncols, out_sb, bias_sb,
                    func=AF.Identity, n_dout=DC, wtag="w", wbufs=2):
            """feature-major projection: out[:, oc, :] = act(sum_ic
            w[ic-chunk, oc-chunk].T @ rhs[:, ic, :] + bias[:, oc])."""
            with tc.tile_pool(name=wtag, bufs=wbufs) as wpool, \
                 tc.tile_pool(name=wtag + "p", bufs=2, space="PSUM") as pp:
                for hh in range((n_dout * 128) // 512):
                    wh = wpool.tile([128, DC, 512], MDT, tag=wtag)
                    nc.sync.dma_start(out=wh,
                                      in_=w_dram[:, :, hh * 512:(hh + 1) * 512])
                    for o in range(4):
                        oc = hh * 4 + o
                        ps = pp.tile([128, ncols], f32, tag=wtag + "p")
                        for kh in range(ncols // 512):
                            cs = slice(kh * 512, (kh + 1) * 512)
                            for ic in range(DC):
                                nc.tensor.matmul(
                                    ps[:, cs], wh[:, ic, o * 128:(o + 1) * 128],
                                    rhs_sb[:, ic, cs],
                                    start=(ic == 0), stop=(ic == DC - 1))
                        nc.scalar.activation(out_sb[:, oc, :], ps, func,
                                             bias=bias_sb[:, oc:oc + 1],
                                             scale=1.0)

        def v_proj(w_dram, lhs_sb, out_sb, bias_row, wtag, wbufs=2):
            """row-major projection from feature-major lhs (x^T chunks)."""
            with tc.tile_pool(name=wtag, bufs=wbufs) as wpool, \
                 tc.tile_pool(name=wtag + "p", bufs=2, space="PSUM") as pp:
                for dh in range(2):
                    wh = wpool.tile([128, DC, 512], MDT, tag=wtag)
                    nc.sync.dma_start(out=wh,
                                      in_=w_dram[:, :, dh * 512:(dh + 1) * 512])
                    for kc in range(DC):
                        ps = pp.tile([128, 512], f32, tag=wtag + "p")
                        for ic in range(DC):
                            nc.tensor.matmul(
                                ps, lhs_sb[:, ic, kc * 128:(kc + 1) * 128],
                                wh[:, ic, :], start=(ic == 0), stop=False)
                        nc.tensor.matmul(ps, c_ones,
                                         bias_row[0:1, dh * 512:(dh + 1) * 512],
                                         start=False, stop=True)
                        nc.any.tensor_copy(out_sb[:, kc, dh * 512:(dh + 1) * 512],
                                           ps)

        def o_proj(w_dram, avt_sb, res_rm, out_rm_t, bias_row, wtag, wbufs=2):
            """row-major out-projection + residual add into out_rm_t (f32)."""
            with tc.tile_pool(name=wtag, bufs=wbufs) as wpool, \
                 tc.tile_pool(name=wtag + "p", bufs=2, space="PSUM") as pp:
                for dh in range(2):
                    wh = wpool.tile([128, DC, 512], MDT, tag=wtag)
                    nc.sync.dma_start(out=wh,
                                      in_=w_dram[:, :, dh * 512:(dh + 1) * 512])
                    for rc in range(RC):
                        ps = pp.tile([128, 512], f32, tag=wtag + "p")
                        for ic in range(DC):
                            nc.tensor.matmul(
                                ps, avt_sb[:, ic, rc * 128:(rc + 1) * 128],
                                wh[:, ic, :], start=(ic == 0), stop=False)
                        nc.tensor.matmul(ps, c_ones,
                                         bias_row[0:1, dh * 512:(dh + 1) * 512],
                                         start=False, stop=True)
                        nc.vector.tensor_tensor(
                            out=out_rm_t[:, rc, dh * 512:(dh + 1) * 512],
                            in0=ps, in1=res_rm[:, rc, dh * 512:(dh + 1) * 512],
                            op=OP.add)

        def layernorm(li, in_rm, out_ap_fn):
            """row-major LN over free dim D; out_ap_fn(rc) -> DRAM AP dest."""
            with tc.tile_pool(name="ln%d" % li, bufs=4) as sp, \
                 tc.tile_pool(name="lno%d" % li, bufs=2) as op, \
                 tc.tile_pool(name="lngb%d" % li, bufs=1) as gp:
                gt = gp.tile([128, D], f32, tag="g")
                nc.sync.dma_start(out=gt, in_=lng[:, li, :])
                bt = gp.tile([128, D], f32, tag="b")
                nc.sync.dma_start(out=bt, in_=lnb[:, li, :])
                for rc in range(RC):
                    st = sp.tile([128, 2, 6], f32, tag="st")
                    nc.vector.bn_stats(out=st[:, 0, :], in_=in_rm[:, rc, 0:512])
                    nc.vector.bn_stats(out=st[:, 1, :], in_=in_rm[:, rc, 512:1024])
                    mv = sp.tile([128, 2], f32, tag="mv")
                    nc.vector.bn_aggr(out=mv, in_=st)
                    rs = sp.tile([128, 1], f32, tag="rs")
                    nc.scalar.activation(rs, mv[:, 1:2], AF.Sqrt,
                                         bias=c_eps, scale=1.0)
                    nc.vector.reciprocal(rs, rs)
                    lo = op.tile([128, D], f32, tag="lo")
                    nc.vector.tensor_scalar(out=lo, in0=in_rm[:, rc, :],
                                            scalar1=mv[:, 0:1], scalar2=rs,
                                            op0=OP.subtract, op1=OP.mult)
                    nc.vector.tensor_tensor(out=lo, in0=lo, in1=gt, op=OP.mult)
                    nc.vector.tensor_tensor(out=lo, in0=lo, in1=bt, op=OP.add)
                    nc.sync.dma_start(out=out_ap_fn(rc), in_=lo)

        def transpose_rm_to_fm(in_rm, out_fm, tag):
            """[128,RC,D] row-major f32 -> [128,DC,R] feature-major MDT."""
            with tc.tile_pool(name=tag + "p", bufs=2, space="PSUM") as pp:
                for dc in range(DC):
                    pt = pp.tile([128, R], f32, tag=tag + "p")
                    for rc in range(RC):
                        nc.tensor.transpose(pt[:, rc * 128:(rc + 1) * 128],
                                            in_rm[:, rc, dc * 128:(dc + 1) * 128],
                                            c_ident_f)
                    nc.any.tensor_copy(out_fm[:, dc, :], pt)

        def attention(prefix, qt_sb, kt_sb, v_sb, skey, mb_dram, attn_out,
                      avt_sb, rb_dram):
            """512 q-rows x skey keys, 16 heads. Normalized probs -> attn_out;
            A@V (normalized at eviction via DRAM-bounced 1/rowsum broadcast)
            -> avt_sb feature-major."""
            KCH = skey // 128
            with tc.tile_pool(name=prefix + "ae", bufs=5) as ap, \
                 tc.tile_pool(name=prefix + "an", bufs=3) as anp, \
                 tc.tile_pool(name=prefix + "anT", bufs=2) as antp, \
                 tc.tile_pool(name=prefix + "s", bufs=8) as sp, \
                 tc.tile_pool(name=prefix + "rb", bufs=2) as rbp, \
                 tc.tile_pool(name=prefix + "rbc", bufs=2) as rbcp, \
                 tc.tile_pool(name=prefix + "pl", bufs=2, space="PSUM") as pl, \
                 tc.tile_pool(name=prefix + "pt", bufs=2, space="PSUM") as pt, \
                 tc.tile_pool(name=prefix + "pv", bufs=2, space="PSUM") as pav:
                mb_sb = None
                if mb_dram is not None:
                    mb_sb = ap.tile([128, RC, S], bf16, tag=prefix + "mb",
                                    name=prefix + "_mb", bufs=1)
                    nc.sync.dma_start(out=mb_sb, in_=mb_dram[:, :, :])
                for h in range(H):
                    bp = (h % 2) * 64
                    fc = h // 2
                    rb = rbp.tile([128, RC], f32, tag=prefix + "rb")
                    ae_tiles = []
                    for qt in range(RC):
                        ps = pl.tile([128, skey], f32, tag=prefix + "L")
                        for kh in range(skey // 512):
                            cs = slice(kh * 512, (kh + 1) * 512)
                            nc.tensor.matmul(
                                ps[:, cs],
                                qt_sb[bp:bp + 64, fc, qt * 128:(qt + 1) * 128],
                                kt_sb[bp:bp + 64, fc, cs],
                                start=True, stop=False)
                            if mb_sb is not None:
                                nc.tensor.matmul(ps[:, cs], c_ident_b,
                                                 mb_sb[:, qt, cs],
                                                 start=False, stop=True)
                            else:
                                nc.tensor.matmul(ps[:, cs], c_ones,
                                                 c_pm[0:1, cs],
                                                 start=False, stop=True)
                        ae = ap.tile([128, skey], ATT_DT, tag=prefix + "ae")
                        ssum = sp.tile([128, 1], f32, tag=prefix + "sm")
                        nc.scalar.activation(ae, ps, AF.Exp, accum_out=ssum)
                        nc.vector.reciprocal(rb[:, qt:qt + 1], ssum)
                        an = anp.tile([128, skey], ATT_DT, tag=prefix + "an")
                        nc.vector.tensor_scalar(out=an, in0=ae,
                                                scalar1=rb[:, qt:qt + 1],
                                                scalar2=None, op0=OP.mult)
                        nc.sync.dma_start(
                            out=attn_out[h, qt * 128:(qt + 1) * 128, :],
                            in_=(an if KDT == "bf16" else an.bitcast(f32)))
                        ae_tiles.append(ae)
                    # 1/rowsum -> DRAM (transposed) -> [64, R] broadcast
                    nc.sync.dma_start(
                        out=bass.AP(tensor=rb_dram, offset=h * R,
                                    ap=[[1, 128], [128, RC]]),
                        in_=rb)
                    rbc = rbcp.tile([64, R], f32, tag=prefix + "rbc")
                    nc.sync.dma_start(
                        out=rbc,
                        in_=bass.AP(tensor=rb_dram, offset=h * R,
                                    ap=[[0, 64], [1, R]]))
                    anT = antp.tile([128, KCH, R], ATT_DT, tag=prefix + "anT")
                    for kc in range(KCH):
                        pst = pt.tile([128, R], ATT_DT, tag=prefix + "T")
                        for qt in range(RC):
                            nc.tensor.transpose(
                                pst[:, qt * 128:(qt + 1) * 128],
                                ae_tiles[qt][:, kc * 128:(kc + 1) * 128],
                                c_ident_m)
                        nc.vector.tensor_copy(anT[:, kc, :], pst)
                    pa = pav.tile([64, R], f32, tag=prefix + "av")
                    for kc in range(KCH):
                        nc.tensor.matmul(pa, v_sb[:, kc, h * 64:(h + 1) * 64],
                                         anT[:, kc, :],
                                         start=(kc == 0), stop=(kc == KCH - 1))
                    nc.vector.tensor_tensor(out=avt_sb[bp:bp + 64, fc, :],
                                            in0=pa, in1=rbc, op=OP.mult)

        # ============ phases (strict LIFO pool nesting) ============
        pers = top.enter_context(tc.tile_pool(name="pers", bufs=1))

        # ---- phase 1+2: self attention ----
        with tc.tile_pool(name="sa", bufs=1) as sa:
            kt_sb = sa.tile([128, DC, S], MDT, tag="kt")
            qt_t = sa.tile([128, DC, R], MDT, tag="qt")
            v_sb = sa.tile([128, DC, D], MDT, tag="v")
            with tc.tile_pool(name="xp1", bufs=1) as xp1:
                xt_sb = xp1.tile([128, DC, S], MDT, tag="xt")
                nc.sync.dma_start(out=xt_sb, in_=xt[:, :, :])
                fm_proj(w["mwk"], xt_sb, S, kt_sb, c_bk_m, wtag="wk", wbufs=2)
                v_proj(w["mwv"], xt_sb, v_sb, c_bv_m, "wv", wbufs=2)
            with tc.tile_pool(name="xp2", bufs=1) as xp2:
                xtr_sb = xp2.tile([128, DC, R], MDT, tag="xtr")
                nc.sync.dma_start(out=xtr_sb, in_=xtr[:, :, :])
                fm_proj(w["mwq"], xtr_sb, R, qt_t, c_bq_m, wtag="wq", wbufs=2)
            avt_m = pers.tile([128, DC, R], MDT, tag="avt", name="avt_m")
            if KPH >= 2:
                attention("sa", qt_t, kt_sb, v_sb, S, mb, m_attn_o, avt_m, rb_m)

        # ---- phases 3+4 (overlapped): self out-proj + LN1 | cross proj ----
        if KPH >= 3:
            with tc.tile_pool(name="sc", bufs=1) as sc:
                kt_c = sc.tile([128, DC, T], MDT, tag="ktc")
                qt_c = sc.tile([128, DC, R], MDT, tag="qtc")
                v_c = sc.tile([128, DC, D], MDT, tag="vc")
                with tc.tile_pool(name="ep", bufs=1) as ep:
                    enct_sb = ep.tile([128, DC, T], MDT, tag="enct")
                    nc.sync.dma_start(out=enct_sb, in_=enct[:, :, :])
                    with tc.tile_pool(name="m1p", bufs=1) as m1p:
                        c_xrow = m1p.tile([128, RC, D], f32, tag="xrow")
                        nc.sync.dma_start(out=c_xrow, in_=xrow[:, :, :])
                        m1 = m1p.tile([128, RC, D], f32, tag="m1")
                        o_proj(w["mwo"], avt_m, c_xrow, m1, c_bo_m, "wo", wbufs=1)
                        layernorm(0, m1, lambda rc: out1_d[:, rc, :])
                    if KPH >= 4:
                        fm_proj(w["cwk"], enct_sb, T, kt_c, c_bk_c,
                                wtag="wkc", wbufs=2)
                        v_proj(w["cwv"], enct_sb, v_c, c_bv_c, "wvc", wbufs=2)
                if KPH >= 4:
                    with tc.tile_pool(name="o1tp", bufs=1) as o1tp:
                        o1l = o1tp.tile([128, RC, D], f32, tag="o1l")
                        nc.sync.dma_start(out=o1l, in_=out1_d[:, :, :])
                        o1t = o1tp.tile([128, DC, R], MDT, tag="out1t")
                        transpose_rm_to_fm(o1l, o1t, "t1")
                        fm_proj(w["cwq"], o1t, R, qt_c, c_bq_c,
                                wtag="wqc", wbufs=2)
                avt_c = pers.tile([128, DC, R], MDT, tag="avt", name="avt_c")
                if KPH >= 5:
                    attention("ca", qt_c, kt_c, v_c, T, None, c_attn_o,
                              avt_c, rb_c)

        # ---- phases 6+7 (overlapped): cross out-proj + LN2 | FFN ----
        if KPH >= 6:
            with tc.tile_pool(name="ffp", bufs=1) as ffp:
                o2l = ffp.tile([128, RC, D], f32, tag="o2l")
                h1t = ffp.tile([128, HC, R], MDT, tag="h1t")
                m3 = ffp.tile([128, RC, D], f32, tag="m3")
                with tc.tile_pool(name="m2p", bufs=1) as m2p:
                    o1l2 = m2p.tile([128, RC, D], f32, tag="o1l2")
                    nc.sync.dma_start(out=o1l2, in_=out1_d[:, :, :])
                    m2 = m2p.tile([128, RC, D], f32, tag="m2")
                    o_proj(w["cwo"], avt_c, o1l2, m2, c_bo_c, "woc")
                    layernorm(1, m2, lambda rc: out2_d[:, rc, :])
                if KPH >= 7:
                    nc.sync.dma_start(out=o2l, in_=out2_d[:, :, :])
                    with tc.tile_pool(name="o2tp", bufs=1) as o2tp:
                        out2t = o2tp.tile([128, DC, R], MDT, tag="out2t")
                        transpose_rm_to_fm(o2l, out2t, "t2")
                        fm_proj(w["w1"], out2t, R, h1t, c_b1, func=AF.Relu,
                                n_dout=HC, wtag="w1", wbufs=2)
                    with tc.tile_pool(name="w2p", bufs=2) as w2p, \
                         tc.tile_pool(name="pf", bufs=1, space="PSUM") as pf:
                        for dh in range(2):
                            pstiles = [pf.tile([128, 512], f32, tag="pf%d" % rc,
                                               name="pf%d_%d" % (dh, rc))
                                       for rc in range(RC)]
                            for wc in range(4):
                                w2c = w2p.tile([128, 8, 512], MDT, tag="w2c")
                                nc.sync.dma_start(
                                    out=w2c,
                                    in_=w["w2"][:, wc * 8:(wc + 1) * 8,
                                                dh * 512:(dh + 1) * 512])
                                for rc in range(RC):
                                    ps = pstiles[rc]
                                    for i in range(8):
                                        hc = wc * 8 + i
                                        nc.tensor.matmul(
                                            ps,
                                            h1t[:, hc, rc * 128:(rc + 1) * 128],
                                            w2c[:, i, :],
                                            start=(hc == 0), stop=False)
                                    if wc == 3:
                                        nc.tensor.matmul(
                                            ps, c_ones,
                                            c_b2[0:1, dh * 512:(dh + 1) * 512],
                                            start=False, stop=True)
                                        nc.vector.tensor_tensor(
                                            out=m3[:, rc, dh * 512:(dh + 1) * 512],
                                            in0=ps,
                                            in1=o2l[:, rc, dh * 512:(dh + 1) * 512],
                                            op=OP.add)
                    layernorm(2, m3,
                              lambda rc: out3_o[rc * 128:(rc + 1) * 128, :])

    nc.finalize()
    return nc


def _fm(a):
    """[din, dout] -> [128, din/128, dout] (partition = din within chunk)"""
    din_, dout_ = a.shape
    return np.ascontiguousarray(
        a.reshape(din_ // 128, 128, dout_).transpose(1, 0, 2))


def _rm(a):
    rows, d_ = a.shape
    return np.ascontiguousarray(a.reshape(rows // 128, 128, d_).transpose(1, 0, 2))


def _prep_inputs(inputs):
    mdt = ml_dtypes.bfloat16 if KDT == "bf16" else np.float32
    f = lambda k: np.asarray(inputs[k], dtype=np.float32)
    x = f('x')
    enc = f('enc_output')
    lam = f('look_ahead_mask')
    pm = f('padding_mask')
    shared = {
        'mwq': _fm(f('m_wq') / 8.0).astype(mdt), 'mwk': _fm(f('m_wk')).astype(mdt),
        'mwv': _fm(f('m_wv')).astype(mdt), 'mwo': _fm(f('m_wo')).astype(mdt),
        'cwq': _fm(f('c_wq') / 8.0).astype(mdt), 'cwk': _fm(f('c_wk')).astype(mdt),
        'cwv': _fm(f('c_wv')).astype(mdt), 'cwo': _fm(f('c_wo')).astype(mdt),
        'w1': _fm(f('ffn_w1')).astype(mdt), 'w2': _fm(f('ffn_w2')).astype(mdt),
        'bq_m': np.ascontiguousarray((f('m_bq') / 8.0).reshape(DC, 128).T),
        'bk_m': np.ascontiguousarray(f('m_bk').reshape(DC, 128).T),
        'bq_c': np.ascontiguousarray((f('c_bq') / 8.0).reshape(DC, 128).T),
        'bk_c': np.ascontiguousarray(f('c_bk').reshape(DC, 128).T),
        'b1': np.ascontiguousarray(f('ffn_b1').reshape(HC, 128).T),
        'bv_m': f('m_bv').reshape(1, D).astype(mdt),
        'bo_m': f('m_bo').reshape(1, D).astype(mdt),
        'bv_c': f('c_bv').reshape(1, D).astype(mdt),
        'bo_c': f('c_bo').reshape(1, D).astype(mdt),
        'b2': f('ffn_b2').reshape(1, D).astype(mdt),
        'lng': np.ascontiguousarray(np.broadcast_to(
            np.stack([f('ln1_g'), f('ln2_g'), f('ln3_g')], 0)[None],
            (128, 3, D))),
        'lnb': np.ascontiguousarray(np.broadcast_to(
            np.stack([f('ln1_b'), f('ln2_b'), f('ln3_b')], 0)[None],
            (128, 3, D))),
    }
    in_maps = []
    for c in range(NC):
        b = c // 2
        off = (c % 2) * R
        m = dict(shared)
        m['xt'] = _fm(np.ascontiguousarray(x[b].T)).astype(mdt)
        m['xtr'] = _fm(np.ascontiguousarray(x[b, off:off + R].T)).astype(mdt)
        m['xrow'] = _rm(x[b, off:off + R])
        m['enct'] = _fm(np.ascontiguousarray(enc[b].T)).astype(mdt)
        mbr = (lam[0, 0, off:off + R, :] * np.float32(-1e9)).astype(
            ml_dtypes.bfloat16)
        m['mb'] = _rm(mbr)
        m['pmr'] = np.ascontiguousarray(
            (pm[b, 0, 0, :] * np.float32(-1e9)).reshape(1, T)).astype(mdt)
        in_maps.append(m)
    return in_maps


def _run(inputs, trace=False):
    if 'nc' not in _cache:
        _cache['nc'] = _build()
    nc = _cache['nc']
    in_maps = _prep_inputs(inputs)
    res = run_bass_kernel_spmd(nc, in_maps, list(range(NC)), trace=trace)
    out3 = np.zeros((B, S, D), np.float32)
    m_attn = np.zeros((B, H, S, S), np.float32)
    c_attn = np.zeros((B, H, S, T), np.float32)
    for c in range(NC):
        b = c // 2
        off = (c % 2) * R
        r = res.results[c]
        out3[b, off:off + R] = r['out3_o']
        m_attn[b, :, off:off + R, :] = np.asarray(r['m_attn_o'], np.float32)
        c_attn[b, :, off:off + R, :] = np.asarray(r['c_attn_o'], np.float32)
    return (out3, m_attn, c_attn), res


def kernel(**inputs):
    outs, _ = _run(inputs, trace=False)
    return outs
